# revision 8
# baseline (speedup 1.0000x reference)
"""Trainium2 Bass kernel for Bidirectional Temporal Self Attention.

out = x * (g1+g2+g3) where each g_b = sigmoid(rank1-attention(conv1d(mean_CHW(x)))).

Sharding: pure data parallel over batch N (16) across 8 cores (2 each).
Per core: phase A streams x computing per-(n,t) means, phase B does the tiny
[2,30] conv + rank-1 attention on-chip, phase C streams x again multiplying by
the broadcast per-(n,t) scale.
"""
import numpy as np

import concourse.bass as bass
from concourse import bacc
import concourse.tile as tile
from concourse import mybir
from concourse import bass_utils

N, C, T, H, W = 16, 128, 30, 64, 44
HW = H * W                 # 2816
NCORES = 8
NP_ = N // NCORES          # 2 batch items per core
TB = 5                     # t-block per streamed tile
NBLK = T // TB             # 6 blocks per batch item
F32 = mybir.dt.float32
X_AX = mybir.AxisListType.X
MUL = mybir.AluOpType.mult
ADD = mybir.AluOpType.add

WSPECS = [("wq1", 3), ("wk1", 3), ("wv1", 3),
          ("wq2", 5), ("wk2", 5), ("wv2", 5),
          ("wq3", 7), ("wk3", 7), ("wv3", 7)]


def _emit_conv(nc, dst, y2, w_sb, k):
    """dst[2,30] = SAME cross-correlation of y2[2,30] with w_sb[2,k] taps."""
    p = (k - 1) // 2
    nc.vector.memset(dst[:], 0.0)
    for m in range(k):
        s = m - p
        lo, hi = max(0, -s), min(T, T - s)
        nc.vector.scalar_tensor_tensor(
            out=dst[:, lo:hi],
            in0=y2[:, lo + s:hi + s],
            scalar=w_sb[:, m:m + 1],
            in1=dst[:, lo:hi],
            op0=MUL,
            op1=ADD,
        )


def build_bass():
    nc = bacc.Bacc("TRN2")
    x = nc.declare_dram_parameter("x", [NP_, C, T, H, W], F32, isOutput=False)
    wh = {name: nc.declare_dram_parameter(name, [1, 1, k], F32, isOutput=False)
          for name, k in WSPECS}
    out = nc.declare_dram_parameter("out", [NP_, C, T, H, W], F32, isOutput=True)

    xv = x[:].rearrange("n c t h w -> n c t (h w)")
    ov = out[:].rearrange("n c t h w -> n c t (h w)")

    with tile.TileContext(nc) as tc:
        with (
            tc.tile_pool(name="data", bufs=3) as data_pool,
            tc.tile_pool(name="small", bufs=1) as small,
            tc.tile_pool(name="psum", bufs=2, space="PSUM") as psum,
        ):
            # --- constants / weights (issued first, overlap with phase A) ---
            w_sb = {}
            for name, k in WSPECS:
                wt = small.tile([2, k], F32, tag=f"w_{name}")
                src = wh[name][:].rearrange("a b k -> a (b k)")
                nc.sync.dma_start(wt[0:1, :], src)
                nc.sync.dma_start(wt[1:2, :], src)
                w_sb[name] = wt
            ones128 = small.tile([128, 1], F32, tag="ones128")
            nc.vector.memset(ones128[:], 1.0)
            ones2 = small.tile([2, 128], F32, tag="ones2")
            nc.vector.memset(ones2[:], 1.0)

            # --- phase A: stream x, per-C partial sums P[128, 60] ---
            P = small.tile([128, NP_ * T], F32, tag="P")
            retained_key = (NP_ - 1, NBLK - 1)
            retained = None
            for n in range(NP_):
                for b in range(NBLK):
                    tl = data_pool.tile([C, TB, HW], F32, tag="data")
                    nc.sync.dma_start(tl[:], xv[n, :, b * TB:(b + 1) * TB, :])
                    j0 = n * T + b * TB
                    nc.vector.reduce_sum(P[:, j0:j0 + TB], tl[:], axis=X_AX)
                    if (n, b) == retained_key:
                        retained = tl

            # --- phase B: tiny conv + rank-1 attention ---
            y_psum = psum.tile([1, NP_ * T], F32, tag="y_psum")
            nc.tensor.matmul(y_psum[:], lhsT=ones128[:], rhs=P[:], start=True, stop=True)
            y_row = small.tile([1, NP_ * T], F32, tag="y_row")
            nc.scalar.mul(y_row[:], y_psum[:], 1.0 / float(C * HW))
            y2 = small.tile([NP_, T], F32, tag="y2")
            nc.sync.dma_start(y2[:], y_row[0:1, :])  # [1,60] -> [2,30] relayout

            gsum = small.tile([NP_, T], F32, tag="gsum")
            for bi, (qn, kn, vn) in enumerate(
                    [("wq1", "wk1", "wv1"), ("wq2", "wk2", "wv2"), ("wq3", "wk3", "wv3")]):
                ksz = dict(WSPECS)[qn]
                q_t = small.tile([NP_, T], F32, tag=f"q{bi}")
                k_t = small.tile([NP_, T], F32, tag=f"k{bi}")
                v_t = small.tile([NP_, T], F32, tag=f"v{bi}")
                _emit_conv(nc, q_t, y2, w_sb[qn], ksz)
                _emit_conv(nc, k_t, y2, w_sb[kn], ksz)
                _emit_conv(nc, v_t, y2, w_sb[vn], ksz)

                blkq = small.tile([NP_, NP_ * T], F32, tag=f"blkq{bi}")
                nc.vector.memset(blkq[:], 0.0)
                nc.vector.tensor_copy(blkq[0:1, 0:T], q_t[0:1, :])
                nc.sync.dma_start(blkq[1:2, T:2 * T], q_t[1:2, :])

                S = psum.tile([NP_ * T, T], F32, tag="S")
                nc.tensor.matmul(S[:], lhsT=blkq[:], rhs=k_t[:], start=True, stop=True)

                mx = small.tile([NP_ * T, 1], F32, tag=f"mx{bi}")
                nc.vector.reduce_max(mx[:], S[:], axis=X_AX)
                nmx = small.tile([NP_ * T, 1], F32, tag=f"nmx{bi}")
                nc.vector.tensor_scalar_mul(nmx[:], mx[:], -1.0)
                E = small.tile([NP_ * T, T], F32, tag=f"E{bi}")
                nc.scalar.activation(E[:], S[:], mybir.ActivationFunctionType.Exp,
                                     bias=nmx[:], scale=1.0)
                Z = small.tile([NP_ * T, 1], F32, tag=f"Z{bi}")
                nc.vector.reduce_sum(Z[:], E[:], axis=X_AX)
                R = small.tile([NP_ * T, 1], F32, tag=f"R{bi}")
                nc.vector.reciprocal(R[:], Z[:])
                nc.vector.tensor_scalar_mul(E[:], E[:], R[:])

                blkv = small.tile([NP_ * T, NP_], F32, tag=f"blkv{bi}")
                nc.vector.memset(blkv[:], 0.0)
                nc.sync.dma_start(blkv[0:T, 0:1], v_t[0:1, :])
                nc.sync.dma_start(blkv[T:2 * T, 1:2], v_t[1:2, :])

                outp = psum.tile([NP_, T], F32, tag="outp")
                nc.tensor.matmul(outp[:], lhsT=blkv[:], rhs=E[:], start=True, stop=True)

                if bi == 0:
                    nc.scalar.activation(gsum[:], outp[:],
                                         mybir.ActivationFunctionType.Sigmoid)
                else:
                    g_b = small.tile([NP_, T], F32, tag=f"g{bi}")
                    nc.scalar.activation(g_b[:], outp[:],
                                         mybir.ActivationFunctionType.Sigmoid)
                    nc.vector.tensor_add(gsum[:], gsum[:], g_b[:])

            blkg = small.tile([NP_, NP_ * T], F32, tag="blkg")
            nc.vector.memset(blkg[:], 0.0)
            nc.vector.tensor_copy(blkg[0:1, 0:T], gsum[0:1, :])
            nc.sync.dma_start(blkg[1:2, T:2 * T], gsum[1:2, :])
            sc_psum = psum.tile([C, NP_ * T], F32, tag="sc_psum")
            nc.tensor.matmul(sc_psum[:], lhsT=ones2[:], rhs=blkg[:], start=True, stop=True)
            scales = small.tile([C, NP_ * T], F32, tag="scales")
            nc.vector.tensor_copy(scales[:], sc_psum[:])

            # --- phase C: stream x again, scale, store ---
            order = [retained_key] + [(n, b) for n in range(NP_) for b in range(NBLK)
                                      if (n, b) != retained_key]
            for (n, b) in order:
                if (n, b) == retained_key:
                    tl = retained
                else:
                    tl = data_pool.tile([C, TB, HW], F32, tag="data")
                    nc.sync.dma_start(tl[:], xv[n, :, b * TB:(b + 1) * TB, :])
                for i in range(TB):
                    j = n * T + b * TB + i
                    nc.vector.tensor_scalar_mul(tl[:, i, :], tl[:, i, :],
                                                scales[:, j:j + 1])
                nc.scalar.dma_start(ov[n, :, b * TB:(b + 1) * TB, :], tl[:])

    nc.compile()
    return nc


_NC_CACHE = None


def _get_nc():
    global _NC_CACHE
    if _NC_CACHE is None:
        _NC_CACHE = build_bass()
    return _NC_CACHE


def run(inputs, trace=False, **kw):
    nc = _get_nc()
    x = np.ascontiguousarray(inputs["x"], dtype=np.float32)
    assert x.shape == (N, C, T, H, W), x.shape
    ws = {name: np.ascontiguousarray(inputs[name], dtype=np.float32)
          for name, _ in WSPECS}
    in_maps = []
    for c in range(NCORES):
        m = {"x": x[NP_ * c:NP_ * (c + 1)]}
        m.update(ws)
        in_maps.append(m)
    res = bass_utils.run_bass_kernel_spmd(
        nc, in_maps, core_ids=list(range(NCORES)), trace=trace, **kw)
    outs = np.concatenate([r["out"] for r in res.results], axis=0)
    return outs, res


def kernel(**inputs) -> np.ndarray:
    outs, _ = run(inputs, trace=False)
    return outs
